# revision 1
# baseline (speedup 1.0000x reference)
"""AdaptiveGraphConvolution on 8 TRN2 NeuronCores — v3 (streamed gather,
on-device A build).

Math: out = sum_l m_l * segment_sum_l(val * x[col] by row) @ W_l + bias
Reordered: aggregate in input-feature space first (per graph), project after:
    g_l[r, :] = sum_{e in graph l, row_e = r} val_e * x[col_e, :]
    out[r, :] = sum_l g_l[r, :] @ (m_l * W_l) + bias

v1 gathered x rows on-device via gpsimd dma_gather (GPSIMD 94% busy —
bottleneck). v2 streamed host-materialized G + A chunk matrices via HWDGE
(hit the 358 GB/s per-NC HBM ceiling, DMA 96% busy). v3 cuts bytes: the A
selection matrices (128x32 per chunk, ~32MB) are built ON-DEVICE by DVE from
a compact per-slot (dcol, val) stream (4B/slot, ~2MB):
    A[e, ci, d] = val[e, ci] * (iota32[d] == dcol[e, ci])
via two tensor_tensor ops with stride-0 broadcast APs.

Sharding: destination rows across 8 cores (6250 rows each), 49 blocks of
128 rows. Edges grouped by (block, graph l, 32-row subblock s); each group
padded to whole 128-edge chunks (SPMD-uniform across cores). Per chunk:
  TensorE: gt_psum[:, l, s*32:(s+1)*32] += G_chunk^T @ A_chunk  ([f, d] acc)
Per block: ACT copies gt psum->SBUF bf16 per graph, TensorE projects
out3 += gt_l^T @ W'_l (row-major out), DVE adds bias, sync DMA stores.
"""

import math
import numpy as np
import ml_dtypes

N_NODES = 50000
N_GRAPHS = 4
N_EDGES = 800000
D = 128
N_CORES = 8
ROWS_PER_CORE = N_NODES // N_CORES  # 6250
BLOCK = 128
SUB = 32  # dest columns per A chunk
NSUB = BLOCK // SUB  # 4
NB = math.ceil(ROWS_PER_CORE / BLOCK)  # 49
NG_BUF = 7  # G slab buffering
NA_BUF = 6  # A build buffering (dv stream + built A)


def _host_schedule(x, edge_rows, edge_cols, edge_vals):
    """Build SPMD-uniform chunk schedule + per-core G and (dcol,val) streams."""
    rows = np.asarray(edge_rows).astype(np.int64).ravel()  # graph-major
    cols = np.asarray(edge_cols).astype(np.int64).ravel()
    vals = np.asarray(edge_vals, dtype=np.float32).ravel()
    graph = np.repeat(np.arange(N_GRAPHS, dtype=np.int64), N_EDGES)
    x16 = np.asarray(x, dtype=np.float32).astype(ml_dtypes.bfloat16)

    core = rows // ROWS_PER_CORE
    local = rows - core * ROWS_PER_CORE
    blk = local // BLOCK
    lb = local % BLOCK
    sub = lb // SUB
    dcol = lb % SUB

    gkey = ((core * NB + blk) * N_GRAPHS + graph) * NSUB + sub
    n_groups = N_CORES * NB * N_GRAPHS * NSUB
    cnt = np.bincount(gkey, minlength=n_groups).reshape(N_CORES, NB, N_GRAPHS, NSUB)
    # 64-slot granularity: per group, full 128-slot chunks plus at most one
    # 64-slot tail; tails pair up two-per-stripe (partitions 0-63 / 64-127)
    n64 = np.maximum(1, np.ceil(cnt.max(axis=0) / 64).astype(np.int64))  # [NB,4,4]
    nfull = n64 // 2
    htail = (n64 % 2).astype(np.int64)

    # per-block stripe layout: all full chunks (l,s order), then tail stripes
    NGRP = N_GRAPHS * NSUB
    nfull_f = nfull.reshape(NB, NGRP)
    htail_f = htail.reshape(NB, NGRP)
    SF_b = nfull_f.sum(axis=1)
    T_b = htail_f.sum(axis=1)
    C_b = SF_b + (T_b + 1) // 2  # stripes per block
    total_chunks = int(C_b.sum())
    off_b = np.zeros(NB + 1, dtype=np.int64)
    off_b[1:] = np.cumsum(C_b)
    # full-chunk base stripe per group
    innerF = np.zeros_like(nfull_f)
    innerF[:, 1:] = np.cumsum(nfull_f, axis=1)[:, :-1]
    fullbase = off_b[:NB, None] + innerF  # [NB, NGRP]
    # tail stripe + half per group (tails packed 2 per stripe, l,s order)
    tailrank = np.cumsum(htail_f, axis=1) - htail_f  # rank among tails
    tailstripe = off_b[:NB, None] + SF_b[:, None] + tailrank // 2
    tailhalf = tailrank % 2
    # group schedule for _build_nc
    groups = {
        "nfull": nfull_f,
        "fullbase": fullbase,
        "tailstripe": np.where(htail_f > 0, tailstripe, -1),
        "tailhalf": tailhalf,
    }

    order = np.argsort(gkey, kind="stable")
    sorted_key = gkey[order]
    grp_start = np.searchsorted(sorted_key, np.arange(n_groups), side="left")
    rank_sorted = np.arange(len(order)) - grp_start[sorted_key]
    rank = np.empty_like(rank_sorted)
    rank[order] = rank_sorted

    gi = graph * NSUB + sub  # group index within block
    nf_e = nfull_f[blk, gi]
    is_full = rank < 128 * nf_e
    chunk = np.where(
        is_full,
        fullbase[blk, gi] + rank // 128,
        tailstripe[blk, gi],
    )
    slot = np.where(
        is_full,
        rank % 128,
        64 * tailhalf[blk, gi] + (rank - 128 * nf_e),
    )

    g_arrs, dv_arrs = [], []
    for s_core in range(N_CORES):
        m = core == s_core
        G = np.zeros((128, total_chunks, D), dtype=ml_dtypes.bfloat16)
        G[slot[m], chunk[m], :] = x16[cols[m]]
        g_arrs.append(G.reshape(128, total_chunks * D))
        # dv stream: per block, [dcol(cb) | val(cb)] as bf16. Pad slots get
        # dcol = -1 (never matches iota 0..31).
        DV = np.full((128, total_chunks, 2), -1.0, dtype=ml_dtypes.bfloat16)
        DV[:, :, 1] = 0.0
        DV[slot[m], chunk[m], 0] = dcol[m].astype(ml_dtypes.bfloat16)
        DV[slot[m], chunk[m], 1] = vals[m].astype(ml_dtypes.bfloat16)
        # per-block layout: dcols of the block's chunks, then vals
        dv = np.empty((128, total_chunks * 2), dtype=ml_dtypes.bfloat16)
        for b in range(NB):
            o, cb = off_b[b], C_b[b]
            dv[:, 2 * o : 2 * o + cb] = DV[:, o : o + cb, 0]
            dv[:, 2 * o + cb : 2 * (o + cb)] = DV[:, o : o + cb, 1]
        dv_arrs.append(dv)

    return {
        "groups": groups,
        "C_b": C_b,
        "total_chunks": total_chunks,
        "g_arrs": g_arrs,
        "dv_arrs": dv_arrs,
    }


def _build_nc(groups, C_b, total_chunks):
    import concourse.bacc as bacc
    import concourse.bass as bass
    import concourse.mybir as mybir
    import contextlib

    Cmax = int(C_b.max())
    off = np.zeros(NB + 1, dtype=np.int64)
    off[1:] = np.cumsum(C_b)
    row_cnt = [min(BLOCK, ROWS_PER_CORE - BLOCK * b) for b in range(NB)]

    nc = bacc.Bacc("TRN2")
    bf16 = mybir.dt.bfloat16
    f32 = mybir.dt.float32

    g_d = nc.declare_dram_parameter("gmat", [128, total_chunks * D], bf16, isOutput=False)
    dv_d = nc.declare_dram_parameter("dvs", [128, total_chunks * 2], bf16, isOutput=False)
    wp_d = nc.declare_dram_parameter("wp", [128, N_GRAPHS * D], bf16, isOutput=False)
    # const2 = [ones(D) | bias(D)] on one partition, for the K=1 bias matmul
    const2_d = nc.declare_dram_parameter("const2", [1, 2 * D], bf16, isOutput=False)
    iota_d = nc.declare_dram_parameter("iota32", [128, SUB], bf16, isOutput=False)
    out_d = nc.declare_dram_parameter("out", [ROWS_PER_CORE, D], f32, isOutput=True)

    with contextlib.ExitStack() as ctx:
        block = ctx.enter_context(nc.Block())
        g_bufs = [
            ctx.enter_context(nc.sbuf_tensor(f"g{i}", [128, Cmax * D], bf16))
            for i in range(NG_BUF)
        ]
        dv_bufs = [
            ctx.enter_context(nc.sbuf_tensor(f"dv{i}", [128, Cmax * 2], bf16))
            for i in range(NA_BUF)
        ]
        a_bufs = [
            ctx.enter_context(nc.sbuf_tensor(f"a{i}", [128, Cmax, SUB], bf16))
            for i in range(NA_BUF)
        ]
        wp_sb = ctx.enter_context(nc.sbuf_tensor("wp_sb", [128, N_GRAPHS * D], bf16))
        const2_sb = ctx.enter_context(nc.sbuf_tensor("const2_sb", [1, 2 * D], bf16))
        iota_sb = ctx.enter_context(nc.sbuf_tensor("iota_sb", [128, SUB], bf16))
        gt_sb = ctx.enter_context(nc.sbuf_tensor("gt_sb", [128, 2 * N_GRAPHS * D], bf16))
        stage = ctx.enter_context(nc.sbuf_tensor("stage", [128, 2 * D], f32))
        gt_ps = [
            ctx.enter_context(nc.psum_tensor(f"gt{i}", [128, N_GRAPHS, D], f32))
            for i in range(2)
        ]
        o3_ps = [
            ctx.enter_context(nc.psum_tensor(f"o3{i}", [128, D], f32)) for i in range(2)
        ]
        io = ctx.enter_context(nc.semaphore("io"))
        # one DMA-completion semaphore per buffer slot (multi-DMA increments
        # on a shared sem interleave out of order across SDMA engines)
        g_sems = [ctx.enter_context(nc.semaphore(f"g_sem{i}")) for i in range(NG_BUF)]
        dv_sems = [ctx.enter_context(nc.semaphore(f"dv_sem{i}")) for i in range(NA_BUF)]
        st_sems = [ctx.enter_context(nc.semaphore(f"st_sem{i}")) for i in range(2)]
        abuild_sem = ctx.enter_context(nc.semaphore("abuild_sem"))  # +1 per block
        eq_sem = ctx.enter_context(nc.semaphore("eq_sem"))  # DVE self-sync
        pe_g = ctx.enter_context(nc.semaphore("pe_g"))  # +1 per (l,s) group
        pe_proj = ctx.enter_context(nc.semaphore("pe_proj"))
        act_sem = ctx.enter_context(nc.semaphore("act_sem"))
        dve_sem = ctx.enter_context(nc.semaphore("dve_sem"))  # bias adds

        NGROUP = N_GRAPHS * NSUB  # 16 pe_g increments per block

        def _issue_dv(eng, b):
            cb = int(C_b[b])
            if b >= NA_BUF:
                # dv buffer consumed by DVE A-build op2 of block b-NA_BUF
                eng.wait_ge(abuild_sem, b - NA_BUF + 1)
            eng.dma_start(
                dv_bufs[b % NA_BUF][:, : cb * 2],
                dv_d[:, int(off[b]) * 2 : int(off[b] + cb) * 2],
            ).then_inc(dv_sems[b % NA_BUF], 16)

        def _issue_store(eng, sb):
            # o3_copy(sb) precedes this on the same (ACT) engine: stage ready
            eng.wait_ge(dve_sem, sb + 1)
            eng.dma_start(
                out_d[BLOCK * sb : BLOCK * sb + row_cnt[sb], :],
                stage[: row_cnt[sb], (sb % 2) * D : (sb % 2) * D + D],
            ).then_inc(st_sems[sb % 2], 16)

        @block.sync
        def _(sync):
            # sync's HWDGE ring carries ONLY the big G slabs, back to back
            for b in range(NB):
                cb = int(C_b[b])
                if b >= NG_BUF:
                    sync.wait_ge(pe_g, NGROUP * (b - NG_BUF + 1))
                sync.dma_start(
                    g_bufs[b % NG_BUF][:, : cb * D],
                    g_d[:, int(off[b]) * D : int(off[b] + cb) * D],
                ).then_inc(g_sems[b % NG_BUF], 16)

        def _proj(tensor, pb):
            # projection of block pb, deferred one block so the ACT copy
            # round-trip hides under agg(pb+1)
            for l in range(N_GRAPHS):
                tensor.wait_ge(act_sem, 4 * pb + l + 1)
                if l == 0 and pb >= 2:
                    tensor.wait_ge(dve_sem, pb - 1)  # o3 psum reuse
                tensor.matmul(
                    o3_ps[pb % 2][:, :],
                    gt_sb[:, ((pb % 2) * N_GRAPHS + l) * D : ((pb % 2) * N_GRAPHS + l + 1) * D],
                    wp_sb[:, l * D : (l + 1) * D],
                    start=(l == 0),
                    stop=False,
                )
            # bias via K=1 matmul: o3 += ones^T @ bias_row
            tensor.matmul(
                o3_ps[pb % 2][:, :],
                const2_sb[0:1, 0:D],
                const2_sb[0:1, D : 2 * D],
                start=False,
                stop=True,
            ).then_inc(pe_proj, 1)

        @block.tensor
        def _(tensor):
            tensor.wait_ge(io, 48)
            for b in range(NB):
                tensor.wait_ge(g_sems[b % NG_BUF], 16 * (b // NG_BUF + 1))
                tensor.wait_ge(abuild_sem, b + 1)  # A of block b built
                gbuf = g_bufs[b % NG_BUF]
                abuf = a_bufs[b % NA_BUF]
                for l in range(N_GRAPHS):
                    for s in range(NSUB):
                        gi = l * NSUB + s
                        out_ap = gt_ps[b % 2][:, l, s * SUB : (s + 1) * SUB]
                        nf = int(groups["nfull"][b, gi])
                        fb = int(groups["fullbase"][b, gi]) - int(off[b])
                        ts = int(groups["tailstripe"][b, gi])
                        for i in range(nf):
                            ci = fb + i
                            mm = tensor.matmul(
                                out_ap,
                                gbuf[:, ci * D : (ci + 1) * D],
                                abuf[:, ci, :],
                                start=(i == 0),
                                stop=(i == nf - 1 and ts < 0),
                            )
                        if ts >= 0:
                            ci = ts - int(off[b])
                            po = 64 * int(groups["tailhalf"][b, gi])
                            mm = tensor.matmul(
                                out_ap,
                                gbuf[po : po + 64, ci * D : (ci + 1) * D],
                                abuf[po : po + 64, ci, :],
                                start=(nf == 0),
                                stop=True,
                            )
                        mm.then_inc(pe_g, 1)
                if b >= 1:
                    _proj(tensor, b - 1)
            _proj(tensor, NB - 1)

        def _o3_copy(scalar, pb):
            # stage <- o3 psum (f32), after proj+bias matmuls of block pb
            scalar.wait_ge(pe_proj, pb + 1)
            if pb >= 2:
                scalar.wait_ge(st_sems[pb % 2], 16 * ((pb - 2) // 2 + 1))
            scalar.copy(
                stage[:, (pb % 2) * D : (pb % 2) * D + D],
                o3_ps[pb % 2][:, :],
            ).then_inc(dve_sem, 1)

        @block.scalar
        def _(scalar):
            # ACT owns the second HWDGE ring: init loads, dv loads, stores
            scalar.dma_start(wp_sb[:, :], wp_d[:, :]).then_inc(io, 16)
            scalar.dma_start(const2_sb[:, :], const2_d[:, :]).then_inc(io, 16)
            scalar.dma_start(iota_sb[:, :], iota_d[:, :]).then_inc(io, 16)
            for b in range(min(NA_BUF, NB)):
                _issue_dv(scalar, b)
            for b in range(NB):
                # self-dependent issues first so cross-engine waits don't
                # head-of-line-block them
                if b >= 2:
                    _issue_store(scalar, b - 2)
                if b >= 1:
                    _o3_copy(scalar, b - 1)
                if b + NA_BUF < NB:
                    _issue_dv(scalar, b + NA_BUF)
                for l in range(N_GRAPHS):
                    scalar.wait_ge(pe_g, NGROUP * (b + 1))  # whole gt bank written
                    if b >= 2 and l == 0:
                        scalar.wait_ge(pe_proj, b - 1)  # gt_sb reuse
                    scalar.copy(
                        gt_sb[:, ((b % 2) * N_GRAPHS + l) * D : ((b % 2) * N_GRAPHS + l + 1) * D],
                        gt_ps[b % 2][:, l, :],
                    ).then_inc(act_sem, 1)
            _o3_copy(scalar, NB - 1)
            for sb in (NB - 2, NB - 1):
                _issue_store(scalar, sb)

        def _a_op1(vector, b):
            # eq = (dcol == iota) into a_buf
            cb = int(C_b[b])
            vector.wait_ge(dv_sems[b % NA_BUF], 16 * (b // NA_BUF + 1))
            if b >= NA_BUF:
                # a_buf consumed by PE agg of block b-NA_BUF
                vector.wait_ge(pe_g, NGROUP * (b - NA_BUF + 1))
            dvb = dv_bufs[b % NA_BUF]
            a3 = a_bufs[b % NA_BUF][:, :cb, :]
            dcol_b = dvb[:, :cb].unsqueeze(2).broadcast_to([128, cb, SUB])
            iota_b = iota_sb[:, :].unsqueeze(1).broadcast_to([128, cb, SUB])
            vector.tensor_tensor(
                a3, dcol_b, iota_b, mybir.AluOpType.is_equal
            ).then_inc(eq_sem, 1)

        def _a_op2(vector, b):
            # A = eq * val, in place (op1(b) completion fenced via eq_sem)
            cb = int(C_b[b])
            vector.wait_ge(eq_sem, b + 1)
            dvb = dv_bufs[b % NA_BUF]
            a3 = a_bufs[b % NA_BUF][:, :cb, :]
            val_b = dvb[:, cb : 2 * cb].unsqueeze(2).broadcast_to([128, cb, SUB])
            vector.tensor_tensor(a3, a3, val_b, mybir.AluOpType.mult).then_inc(
                abuild_sem, 1
            )

        @block.vector
        def _(vector):
            # DVE does ONLY the A-build — no coupling to the PE's downstream,
            # so it can run ahead of the PE by the full NA_BUF depth
            vector.wait_ge(io, 48)
            for b in range(NB):
                _a_op1(vector, b)
                if b >= 1:
                    _a_op2(vector, b - 1)
            _a_op2(vector, NB - 1)

    nc.compile()
    return nc


_TRACE = {"on": False, "last": None}


def kernel(x, edge_rows, edge_cols, edge_vals, W, mixing_weight, bias):
    from concourse.bass_utils import run_bass_kernel_spmd

    sched = _host_schedule(x, edge_rows, edge_cols, edge_vals)
    nc = _build_nc(sched["groups"], sched["C_b"], sched["total_chunks"])

    Wp = (np.asarray(mixing_weight, dtype=np.float32)[:, 0, None, None]
          * np.asarray(W, dtype=np.float32))  # [4,128,128]
    wp_arr = np.ascontiguousarray(
        np.transpose(Wp, (1, 0, 2)).reshape(D, N_GRAPHS * D)
    ).astype(ml_dtypes.bfloat16)
    const2 = np.concatenate(
        [np.ones(D, dtype=np.float32), np.asarray(bias, dtype=np.float32)]
    )[None, :].astype(ml_dtypes.bfloat16)
    iota_arr = np.ascontiguousarray(
        np.broadcast_to(np.arange(SUB, dtype=np.float32), (128, SUB))
    ).astype(ml_dtypes.bfloat16)

    in_maps = [
        {
            "gmat": sched["g_arrs"][s],
            "dvs": sched["dv_arrs"][s],
            "wp": wp_arr,
            "const2": const2,
            "iota32": iota_arr,
        }
        for s in range(N_CORES)
    ]

    res = run_bass_kernel_spmd(
        nc, in_maps, core_ids=list(range(N_CORES)), trace=_TRACE["on"]
    )
    _TRACE["last"] = res
    out = np.concatenate(
        [np.asarray(res.results[s]["out"], dtype=np.float32) for s in range(N_CORES)],
        axis=0,
    )
    return out



# revision 12
# speedup vs baseline: 1.2073x; 1.2073x over previous
"""AdaptiveGraphConvolution on 8 TRN2 NeuronCores — v4.

Math: out = sum_l m_l * segment_sum_l(val * x[col] by row) @ W_l + bias
Reordered: aggregate val-scaled rows in input-feature space first (per
graph), project after:
    g_l[r, :] = sum_{e in graph l, row_e = r} val_e * x[col_e, :]
    out[r, :] = sum_l g_l[r, :] @ (m_l * W_l) + bias

G chunks (host-gathered, val pre-scaled, bf16) stream from HBM; the
one-hot selection matrices A are built ON-DEVICE by DVE from a compact
per-slot dcol stream via a single is_equal against an iota constant.

Sharding: destination rows across 8 cores (6250 rows each), 49 blocks of
128 rows. Edges grouped by (block, graph l, 32-row subblock s); each
group has nfull whole 128-slot chunks; the remainders of all 16 groups
of a block pack densely (no rounding) into shared tail stripes whose A
is a [128, 512]-wide one-hot over the whole (l, s, dcol) column space.
Per full chunk:  TensorE: gt_psum[:, l, s*32:+32] += G^T @ A   (N=32)
Per tail stripe: TensorE: gt_psum[:, 0:512]      += G^T @ At  (N=512)
Per block: ACT copies gt psum->SBUF bf16 per graph, TensorE projects
out3 += gt_l^T @ W'_l, +bias via K=1 matmul, ACT stages bf16, DMA out.
"""

import math
import numpy as np
import ml_dtypes

N_NODES = 50000
N_GRAPHS = 4
N_EDGES = 800000
D = 128
N_CORES = 8
ROWS_PER_CORE = N_NODES // N_CORES  # 6250
BLOCK = 128
SUB = 32
NSUB = BLOCK // SUB  # 4
NB = math.ceil(ROWS_PER_CORE / BLOCK)  # 49
NG_BUF = 7  # G slab buffering
NA_BUF = 4  # A build buffering


def _host_schedule(x, edge_rows, edge_cols, edge_vals):
    """SPMD-uniform chunk schedule + per-core G / dcol streams."""
    rows = np.asarray(edge_rows).astype(np.int64).ravel()  # graph-major
    cols = np.asarray(edge_cols).astype(np.int64).ravel()
    vals = np.asarray(edge_vals, dtype=np.float32).ravel()
    graph = np.repeat(np.arange(N_GRAPHS, dtype=np.int64), N_EDGES)
    xf = np.asarray(x, dtype=np.float32)

    core = rows // ROWS_PER_CORE
    local = rows - core * ROWS_PER_CORE
    blk = local // BLOCK
    lb = local % BLOCK
    sub = lb // SUB
    dcol = lb % SUB

    NGRP = N_GRAPHS * NSUB  # 16 groups per block
    gi = graph * NSUB + sub  # group index within block
    gkey = (core * NB + blk) * NGRP + gi
    n_groups = N_CORES * NB * NGRP
    cnt = np.bincount(gkey, minlength=n_groups).reshape(N_CORES, NB, NGRP)
    maxc = cnt.max(axis=0)  # [NB, 16] SPMD-uniform group sizes
    nfull = maxc // 128  # whole 128-slot chunks
    tail_len = maxc - nfull * 128  # 0..127, packed densely into stripes

    # per-block layout: full chunks (group order), then tail stripes
    nf_b = nfull.sum(axis=1)  # [NB]
    tail_off = np.zeros((NB, NGRP), dtype=np.int64)
    tail_off[:, 1:] = np.cumsum(tail_len, axis=1)[:, :-1]
    tail_tot = tail_len.sum(axis=1)  # [NB]
    nt_b = (tail_tot + 127) // 128  # tail stripes per block
    C_b = nf_b + nt_b
    total_chunks = int(C_b.sum())
    off_b = np.zeros(NB + 1, dtype=np.int64)
    off_b[1:] = np.cumsum(C_b)
    fullcum = np.zeros(NB + 1, dtype=np.int64)
    fullcum[1:] = np.cumsum(nf_b)
    tailcum = np.zeros(NB + 1, dtype=np.int64)
    tailcum[1:] = np.cumsum(nt_b)
    total_full = int(fullcum[-1])
    total_tail = int(tailcum[-1])

    innerF = np.zeros_like(nfull)
    innerF[:, 1:] = np.cumsum(nfull, axis=1)[:, :-1]
    fullbase = off_b[:NB, None] + innerF  # [NB, 16] chunk idx of group's fulls
    tailstart = off_b[:NB] + nf_b  # [NB] first tail-stripe chunk idx

    groups = {
        "nfull": nfull,
        "nf_b": nf_b,
        "nt_b": nt_b,
        "fullbase": fullbase,
        "tailstart": tailstart,
        "fullcum": fullcum,
        "tailcum": tailcum,
    }

    # rank of each edge within its (core, block, group)
    order = np.argsort(gkey, kind="stable")
    sorted_key = gkey[order]
    grp_start = np.searchsorted(sorted_key, np.arange(n_groups), side="left")
    rank_sorted = np.arange(len(order)) - grp_start[sorted_key]
    rank = np.empty_like(rank_sorted)
    rank[order] = rank_sorted

    nf_e = nfull[blk, gi]
    is_full = rank < 128 * nf_e
    tpos = tail_off[blk, gi] + (rank - 128 * nf_e)  # tail slot within block
    chunk = np.where(
        is_full,
        fullbase[blk, gi] + rank // 128,
        tailstart[blk] + tpos // 128,
    )
    slot = np.where(is_full, rank % 128, tpos % 128)
    # dv values: full chunks use dcol (0..31); tail stripes use the wide
    # (l, s, dcol) column index 0..511
    dval_tail = (graph * NSUB + sub) * SUB + dcol

    g_arrs, dvf_arrs, dvt_arrs = [], [], []
    for s_core in range(N_CORES):
        m = core == s_core
        G = np.zeros((128, total_chunks, D), dtype=ml_dtypes.bfloat16)
        G[slot[m], chunk[m], :] = (xf[cols[m]] * vals[m][:, None]).astype(
            ml_dtypes.bfloat16
        )
        g_arrs.append(G.reshape(128, total_chunks * D))

        DVF = np.full((128, total_full), -1.0, dtype=ml_dtypes.bfloat16)
        DVT = np.full((128, max(total_tail, 1)), -1.0, dtype=np.float16)
        mf = m & is_full
        mt = m & ~is_full
        # full-chunk global index: fulls precede tails within each block
        fidx = fullcum[blk[mf]] + (chunk[mf] - off_b[blk[mf]])
        DVF[slot[mf], fidx] = dcol[mf].astype(ml_dtypes.bfloat16)
        tidx = tailcum[blk[mt]] + (chunk[mt] - tailstart[blk[mt]])
        DVT[slot[mt], tidx] = dval_tail[mt].astype(np.float16)
        dvf_arrs.append(DVF)
        dvt_arrs.append(DVT)

    return {
        "groups": groups,
        "C_b": C_b,
        "total_chunks": total_chunks,
        "total_full": total_full,
        "total_tail": total_tail,
        "g_arrs": g_arrs,
        "dvf_arrs": dvf_arrs,
        "dvt_arrs": dvt_arrs,
    }


def _build_nc(groups, C_b, total_chunks, total_full, total_tail):
    import concourse.bacc as bacc
    import concourse.bass as bass
    import concourse.mybir as mybir
    import contextlib

    Cmax = int(C_b.max())
    NFmax = int(groups["nf_b"].max())
    NTmax = max(int(groups["nt_b"].max()), 1)
    off = np.zeros(NB + 1, dtype=np.int64)
    off[1:] = np.cumsum(C_b)
    total_tail_alloc = max(total_tail, 1)

    nc = bacc.Bacc("TRN2")
    bf16 = mybir.dt.bfloat16
    f16 = mybir.dt.float16
    f32 = mybir.dt.float32

    g_d = nc.declare_dram_parameter("gmat", [128, total_chunks * D], bf16, isOutput=False)
    dvf_d = nc.declare_dram_parameter("dvf", [128, total_full], bf16, isOutput=False)
    dvt_d = nc.declare_dram_parameter("dvt", [128, total_tail_alloc], f16, isOutput=False)
    # misc: [0:512) wp, [512:544) iota32, partition 0 [544:800) = ones|bias
    misc_d = nc.declare_dram_parameter("misc", [128, 800], bf16, isOutput=False)
    iota32_d = nc.declare_dram_parameter("iota32", [128, SUB], bf16, isOutput=False)
    iota512_d = nc.declare_dram_parameter("iota512", [128, 512], f16, isOutput=False)
    out_d = nc.declare_dram_parameter("out", [NB * BLOCK, D], bf16, isOutput=True)

    with contextlib.ExitStack() as ctx:
        block = ctx.enter_context(nc.Block())
        g_bufs = [
            ctx.enter_context(nc.sbuf_tensor(f"g{i}", [128, Cmax * D], bf16))
            for i in range(NG_BUF)
        ]
        af_bufs = [
            ctx.enter_context(nc.sbuf_tensor(f"af{i}", [128, NFmax, SUB], bf16))
            for i in range(NA_BUF)
        ]
        at_bufs = [
            ctx.enter_context(nc.sbuf_tensor(f"at{i}", [128, NTmax, 512], bf16))
            for i in range(NA_BUF)
        ]
        dvf_sb = ctx.enter_context(nc.sbuf_tensor("dvf_sb", [128, total_full], bf16))
        dvt_sb = ctx.enter_context(nc.sbuf_tensor("dvt_sb", [128, total_tail_alloc], f16))
        misc_sb = ctx.enter_context(nc.sbuf_tensor("misc_sb", [128, 800], bf16))
        iota32_sb = ctx.enter_context(nc.sbuf_tensor("iota32_sb", [128, SUB], bf16))
        iota512_sb = ctx.enter_context(nc.sbuf_tensor("iota512_sb", [128, 512], f16))
        gt_sb = ctx.enter_context(nc.sbuf_tensor("gt_sb", [128, 2 * N_GRAPHS * D], bf16))
        stage = ctx.enter_context(nc.sbuf_tensor("stage", [128, 2 * D], bf16))
        gt_ps = [
            ctx.enter_context(nc.psum_tensor(f"gt{i}", [128, N_GRAPHS * D], f32))
            for i in range(4)
        ]
        o3_ps = [
            ctx.enter_context(nc.psum_tensor(f"o3{i}", [128, D], f32)) for i in range(2)
        ]
        io = ctx.enter_context(nc.semaphore("io"))
        g_sems = [ctx.enter_context(nc.semaphore(f"g_sem{i}")) for i in range(NG_BUF)]
        st_sems = [ctx.enter_context(nc.semaphore(f"st_sem{i}")) for i in range(2)]
        abuild_sem = ctx.enter_context(nc.semaphore("abuild_sem"))  # +1 per block
        pe_blk = ctx.enter_context(nc.semaphore("pe_blk"))  # +1 per block agg
        pe_proj = ctx.enter_context(nc.semaphore("pe_proj"))  # +1 per block proj
        act_sem = ctx.enter_context(nc.semaphore("act_sem"))  # +1 per gt copy
        o3c_sem = ctx.enter_context(nc.semaphore("o3c_sem"))  # +1 per o3 copy

        wp_ap = misc_sb[:, 0:512]
        iota32_ap = misc_sb[:, 512:544]
        ones_ap = misc_sb[0:1, 544 : 544 + D]
        bias_ap = misc_sb[0:1, 544 + D : 544 + 2 * D]

        @block.sync
        def _(sync):
            # sync's HWDGE ring carries ONLY the big G slabs, back to back
            for b in range(NB):
                cb = int(C_b[b])
                if b >= NG_BUF:
                    sync.wait_ge(pe_blk, b - NG_BUF + 1)
                sync.dma_start(
                    g_bufs[b % NG_BUF][:, : cb * D],
                    g_d[:, int(off[b]) * D : int(off[b] + cb) * D],
                ).then_inc(g_sems[b % NG_BUF], 16)

        def _issue_store(eng, sb):
            eng.wait_ge(o3c_sem, sb + 1)
            eng.dma_start(
                out_d[BLOCK * sb : BLOCK * (sb + 1), :],
                stage[:, (sb % 2) * D : (sb % 2) * D + D],
            ).then_inc(st_sems[sb % 2], 16)

        def _proj(tensor, pb):
            # projection of block pb, deferred one block so the ACT copy
            # round-trip hides under agg(pb+1)
            for l in range(N_GRAPHS):
                tensor.wait_ge(act_sem, 4 * pb + l + 1)
                if l == 0 and pb >= 2:
                    tensor.wait_ge(o3c_sem, pb - 1)  # o3 psum reuse
                tensor.matmul(
                    o3_ps[pb % 2][:, :],
                    gt_sb[:, ((pb % 2) * N_GRAPHS + l) * D : ((pb % 2) * N_GRAPHS + l + 1) * D],
                    wp_ap[:, l * D : (l + 1) * D],
                    start=(l == 0),
                    stop=False,
                )
            # bias via K=1 matmul: o3 += ones^T @ bias_row
            tensor.matmul(
                o3_ps[pb % 2][:, :],
                ones_ap,
                bias_ap,
                start=False,
                stop=True,
            ).then_inc(pe_proj, 1)

        @block.tensor
        def _(tensor):
            tensor.wait_ge(io, 64)
            for b in range(NB):
                tensor.wait_ge(g_sems[b % NG_BUF], 16 * (b // NG_BUF + 1))
                tensor.wait_ge(abuild_sem, b + 1)
                gbuf = g_bufs[b % NG_BUF]
                afb = af_bufs[b % NA_BUF]
                atb = at_bufs[b % NA_BUF]
                nt = int(groups["nt_b"][b])
                gt_flat = gt_ps[b % 4]
                # ONE start per bank per block: start=True marks the whole
                # 2KB zero region pending-zero, so later MMs overwrite on
                # first element touch and accumulate after — any additional
                # start would wipe earlier groups' partials for the tails.
                first = True
                last = None
                for l in range(N_GRAPHS):
                    for s in range(NSUB):
                        gidx = l * NSUB + s
                        out_ap = gt_flat[:, l * D + s * SUB : l * D + (s + 1) * SUB]
                        nf = int(groups["nfull"][b, gidx])
                        fb = int(groups["fullbase"][b, gidx]) - int(off[b])
                        for i in range(nf):
                            ci = fb + i
                            last = tensor.matmul(
                                out_ap,
                                gbuf[:, ci * D : (ci + 1) * D],
                                afb[:, ci, :],
                                start=first,
                                stop=False,
                                skip_group_check=True,
                            )
                            first = False
                for t in range(nt):
                    ci = int(groups["tailstart"][b]) - int(off[b]) + t
                    last = tensor.matmul(
                        gt_flat[:, :],
                        gbuf[:, ci * D : (ci + 1) * D],
                        atb[:, t, :],
                        start=first,
                        stop=(t == nt - 1),
                        skip_group_check=True,
                    )
                    first = False
                last.then_inc(pe_blk, 1)
                if b >= 1:
                    _proj(tensor, b - 1)
            _proj(tensor, NB - 1)

        def _o3_copy(scalar, pb):
            # stage <- o3 psum (f32 -> bf16), after proj+bias of block pb
            scalar.wait_ge(pe_proj, pb + 1)
            if pb >= 2:
                scalar.wait_ge(st_sems[pb % 2], 16 * ((pb - 2) // 2 + 1))
            scalar.copy(
                stage[:, (pb % 2) * D : (pb % 2) * D + D],
                o3_ps[pb % 2][:, :],
            ).then_inc(o3c_sem, 1)

        @block.scalar
        def _(scalar):
            # ACT owns the second HWDGE ring: init loads + stores
            scalar.dma_start(misc_sb[:, :], misc_d[:, :]).then_inc(io, 16)
            scalar.dma_start(iota512_sb[:, :], iota512_d[:, :]).then_inc(io, 16)
            scalar.dma_start(dvf_sb[:, :], dvf_d[:, :]).then_inc(io, 16)
            scalar.dma_start(dvt_sb[:, :], dvt_d[:, :]).then_inc(io, 16)
            scalar.dma_start(iota32_sb[:, :], iota32_d[:, :]).then_inc(io, 16)
            for b in range(NB):
                if b >= 2:
                    _issue_store(scalar, b - 2)
                if b >= 1:
                    _o3_copy(scalar, b - 1)
                scalar.wait_ge(pe_blk, b + 1)
                for l in range(N_GRAPHS):
                    if b >= 2 and l == 0:
                        scalar.wait_ge(pe_proj, b - 1)  # gt_sb bank reuse
                    scalar.copy(
                        gt_sb[:, ((b % 2) * N_GRAPHS + l) * D : ((b % 2) * N_GRAPHS + l + 1) * D],
                        gt_ps[b % 4][:, l * D : (l + 1) * D],
                    ).then_inc(act_sem, 1)
            _o3_copy(scalar, NB - 1)
            for sb in (NB - 2, NB - 1):
                _issue_store(scalar, sb)

        @block.vector
        def _(vector):
            # DVE: one-hot A build only
            vector.wait_ge(io, 80)
            fc = groups["fullcum"]
            tc = groups["tailcum"]
            for b in range(NB):
                nf = int(groups["nf_b"][b])
                nt = int(groups["nt_b"][b])
                if b >= NA_BUF:
                    vector.wait_ge(pe_blk, b - NA_BUF + 1)
                afb = af_bufs[b % NA_BUF]
                atb = at_bufs[b % NA_BUF]
                dvf_b = dvf_sb[:, int(fc[b]) : int(fc[b]) + nf]
                a3 = afb[:, :nf, :]
                ieq = vector.tensor_tensor(
                    a3,
                    dvf_b.unsqueeze(2).broadcast_to([128, nf, SUB]),
                    iota32_sb[:, :].unsqueeze(1).broadcast_to([128, nf, SUB]),
                    mybir.AluOpType.is_equal,
                )
                if nt > 0:
                    dvt_b = dvt_sb[:, int(tc[b]) : int(tc[b]) + nt]
                    t3 = atb[:, :nt, :]
                    ieq = vector.tensor_tensor(
                        t3,
                        dvt_b.unsqueeze(2).broadcast_to([128, nt, 512]),
                        iota512_sb[:, :].unsqueeze(1).broadcast_to([128, nt, 512]),
                        mybir.AluOpType.is_equal,
                    )
                ieq.then_inc(abuild_sem, 1)

    nc.compile()
    return nc


_TRACE = {"on": False, "last": None}


def kernel(x, edge_rows, edge_cols, edge_vals, W, mixing_weight, bias):
    from concourse.bass_utils import run_bass_kernel_spmd

    sched = _host_schedule(x, edge_rows, edge_cols, edge_vals)
    nc = _build_nc(
        sched["groups"],
        sched["C_b"],
        sched["total_chunks"],
        sched["total_full"],
        sched["total_tail"],
    )

    Wp = (np.asarray(mixing_weight, dtype=np.float32)[:, 0, None, None]
          * np.asarray(W, dtype=np.float32))  # [4,128,128]
    misc = np.zeros((128, 800), dtype=ml_dtypes.bfloat16)
    misc[:, 0:512] = np.ascontiguousarray(
        np.transpose(Wp, (1, 0, 2)).reshape(D, N_GRAPHS * D)
    ).astype(ml_dtypes.bfloat16)
    misc[:, 512:544] = np.broadcast_to(
        np.arange(SUB, dtype=np.float32), (128, SUB)
    ).astype(ml_dtypes.bfloat16)
    misc[0, 544 : 544 + D] = np.ones(D, dtype=np.float32).astype(ml_dtypes.bfloat16)
    misc[0, 544 + D : 544 + 2 * D] = np.asarray(bias, dtype=np.float32).astype(
        ml_dtypes.bfloat16
    )
    iota512 = np.ascontiguousarray(
        np.broadcast_to(np.arange(512, dtype=np.float32), (128, 512))
    ).astype(np.float16)

    in_maps = [
        {
            "gmat": sched["g_arrs"][s],
            "dvf": sched["dvf_arrs"][s],
            "dvt": sched["dvt_arrs"][s],
            "misc": misc,
            "iota512": iota512,
            "iota32": np.ascontiguousarray(
                np.broadcast_to(np.arange(SUB, dtype=np.float32), (128, SUB))
            ).astype(ml_dtypes.bfloat16),
        }
        for s in range(N_CORES)
    ]

    res = run_bass_kernel_spmd(
        nc, in_maps, core_ids=list(range(N_CORES)), trace=_TRACE["on"]
    )
    _TRACE["last"] = res
    out = np.concatenate(
        [
            np.asarray(res.results[s]["out"][:ROWS_PER_CORE], dtype=np.float32)
            for s in range(N_CORES)
        ],
        axis=0,
    )
    return out


# revision 15
# speedup vs baseline: 1.6079x; 1.3319x over previous
"""AdaptiveGraphConvolution on 8 TRN2 NeuronCores — v4.

Math: out = sum_l m_l * segment_sum_l(val * x[col] by row) @ W_l + bias
Reordered: aggregate val-scaled rows in input-feature space first (per
graph), project after:
    g_l[r, :] = sum_{e in graph l, row_e = r} val_e * x[col_e, :]
    out[r, :] = sum_l g_l[r, :] @ (m_l * W_l) + bias

G chunks (host-gathered, val pre-scaled, bf16) stream from HBM; the
one-hot selection matrices A are built ON-DEVICE by DVE from a compact
per-slot dcol stream via a single is_equal against an iota constant.

Sharding: destination rows across 8 cores (6250 rows each), 49 blocks of
128 rows. Edges grouped by (block, graph l, 32-row subblock s); each
group has nfull whole 128-slot chunks; the remainders of all 16 groups
of a block pack densely (no rounding) into shared tail stripes whose A
is a [128, 512]-wide one-hot over the whole (l, s, dcol) column space.
Per full chunk:  TensorE: gt_psum[:, l, s*32:+32] += G^T @ A   (N=32)
Per tail stripe: TensorE: gt_psum[:, 0:512]      += G^T @ At  (N=512)
Per block: ACT copies gt psum->SBUF bf16 per graph, TensorE projects
out3 += gt_l^T @ W'_l, +bias via K=1 matmul, ACT stages bf16, DMA out.
"""

import math
import numpy as np
import ml_dtypes

N_NODES = 50000
N_GRAPHS = 4
N_EDGES = 800000
D = 128
N_CORES = 8
ROWS_PER_CORE = N_NODES // N_CORES  # 6250
BLOCK = 128
SUB = 32
NSUB = BLOCK // SUB  # 4
NB = math.ceil(ROWS_PER_CORE / BLOCK)  # 49
NG_BUF = 7  # G slab buffering
NA_BUF = 4  # A build buffering


def _host_schedule(x, edge_rows, edge_cols, edge_vals):
    """SPMD-uniform chunk schedule + per-core G / dcol streams."""
    rows = np.asarray(edge_rows).astype(np.int64).ravel()  # graph-major
    cols = np.asarray(edge_cols).astype(np.int64).ravel()
    vals = np.asarray(edge_vals, dtype=np.float32).ravel()
    graph = np.repeat(np.arange(N_GRAPHS, dtype=np.int64), N_EDGES)
    xf = np.asarray(x, dtype=np.float32)

    core = rows // ROWS_PER_CORE
    local = rows - core * ROWS_PER_CORE
    blk = local // BLOCK
    lb = local % BLOCK
    sub = lb // SUB
    dcol = lb % SUB

    NGRP = N_GRAPHS * NSUB  # 16 groups per block
    gi = graph * NSUB + sub  # group index within block
    gkey = (core * NB + blk) * NGRP + gi
    n_groups = N_CORES * NB * NGRP
    cnt = np.bincount(gkey, minlength=n_groups).reshape(N_CORES, NB, NGRP)
    maxc = cnt.max(axis=0)  # [NB, 16] SPMD-uniform group sizes
    nfull = maxc // 128  # whole 128-slot chunks
    tail_len = maxc - nfull * 128  # 0..127, packed densely into stripes

    # per-block layout: full chunks (group order), then tail stripes
    nf_b = nfull.sum(axis=1)  # [NB]
    tail_off = np.zeros((NB, NGRP), dtype=np.int64)
    tail_off[:, 1:] = np.cumsum(tail_len, axis=1)[:, :-1]
    tail_tot = tail_len.sum(axis=1)  # [NB]
    nt_b = (tail_tot + 127) // 128  # tail stripes per block
    C_b = nf_b + nt_b
    total_chunks = int(C_b.sum())
    off_b = np.zeros(NB + 1, dtype=np.int64)
    off_b[1:] = np.cumsum(C_b)
    fullcum = np.zeros(NB + 1, dtype=np.int64)
    fullcum[1:] = np.cumsum(nf_b)
    tailcum = np.zeros(NB + 1, dtype=np.int64)
    tailcum[1:] = np.cumsum(nt_b)
    total_full = int(fullcum[-1])
    total_tail = int(tailcum[-1])

    innerF = np.zeros_like(nfull)
    innerF[:, 1:] = np.cumsum(nfull, axis=1)[:, :-1]
    fullbase = off_b[:NB, None] + innerF  # [NB, 16] chunk idx of group's fulls
    tailstart = off_b[:NB] + nf_b  # [NB] first tail-stripe chunk idx

    # per-stripe contiguous one-hot column window: tail slots are laid out
    # in (l, s) group order, so stripe t's groups span cols [lo, lo+w)
    NTmax = max(int(nt_b.max()), 1)
    tail_lo = np.zeros((NB, NTmax), dtype=np.int64)
    tail_w = np.full((NB, NTmax), 1, dtype=np.int64)
    for b in range(NB):
        for t in range(int(nt_b[b])):
            p0, p1 = 128 * t, min(128 * (t + 1), int(tail_tot[b]))
            gsel = np.nonzero(
                (tail_off[b] < p1) & (tail_off[b] + tail_len[b] > p0)
            )[0]
            if len(gsel) == 0:
                continue
            tail_lo[b, t] = int(gsel.min()) * SUB
            tail_w[b, t] = (int(gsel.max()) + 1) * SUB - tail_lo[b, t]
    Wmax = int(tail_w.max())

    groups = {
        "nfull": nfull,
        "nf_b": nf_b,
        "nt_b": nt_b,
        "fullbase": fullbase,
        "tailstart": tailstart,
        "fullcum": fullcum,
        "tailcum": tailcum,
        "tail_lo": tail_lo,
        "tail_w": tail_w,
        "Wmax": Wmax,
    }

    # rank of each edge within its (core, block, group)
    order = np.argsort(gkey, kind="stable")
    sorted_key = gkey[order]
    grp_start = np.searchsorted(sorted_key, np.arange(n_groups), side="left")
    rank_sorted = np.arange(len(order)) - grp_start[sorted_key]
    rank = np.empty_like(rank_sorted)
    rank[order] = rank_sorted

    nf_e = nfull[blk, gi]
    is_full = rank < 128 * nf_e
    tpos = tail_off[blk, gi] + (rank - 128 * nf_e)  # tail slot within block
    chunk = np.where(
        is_full,
        fullbase[blk, gi] + rank // 128,
        tailstart[blk] + tpos // 128,
    )
    slot = np.where(is_full, rank % 128, tpos % 128)
    # dv values: full chunks use dcol (0..31); tail stripes use the wide
    # (l, s, dcol) column index 0..511
    stripe_e = tpos // 128  # tail stripe index within block (for tail edges)
    lo_e = tail_lo[blk, np.clip(stripe_e, 0, NTmax - 1)]
    dval_tail = (graph * NSUB + sub) * SUB + dcol - lo_e

    g_arrs, dvf_arrs, dvt_arrs = [], [], []
    for s_core in range(N_CORES):
        m = core == s_core
        G = np.zeros((128, total_chunks, D), dtype=ml_dtypes.bfloat16)
        G[slot[m], chunk[m], :] = (xf[cols[m]] * vals[m][:, None]).astype(
            ml_dtypes.bfloat16
        )
        g_arrs.append(G.reshape(128, total_chunks * D))

        DVF = np.full((128, total_full), -1.0, dtype=ml_dtypes.bfloat16)
        DVT = np.full((128, max(total_tail, 1)), -1.0, dtype=np.float16)
        mf = m & is_full
        mt = m & ~is_full
        # full-chunk global index: fulls precede tails within each block
        fidx = fullcum[blk[mf]] + (chunk[mf] - off_b[blk[mf]])
        DVF[slot[mf], fidx] = dcol[mf].astype(ml_dtypes.bfloat16)
        tidx = tailcum[blk[mt]] + (chunk[mt] - tailstart[blk[mt]])
        DVT[slot[mt], tidx] = dval_tail[mt].astype(np.float16)
        dvf_arrs.append(DVF)
        dvt_arrs.append(DVT)

    return {
        "groups": groups,
        "C_b": C_b,
        "total_chunks": total_chunks,
        "total_full": total_full,
        "total_tail": total_tail,
        "g_arrs": g_arrs,
        "dvf_arrs": dvf_arrs,
        "dvt_arrs": dvt_arrs,
    }


def _build_nc(groups, C_b, total_chunks, total_full, total_tail):
    import concourse.bacc as bacc
    import concourse.bass as bass
    import concourse.mybir as mybir
    import contextlib

    Cmax = int(C_b.max())
    NFmax = int(groups["nf_b"].max())
    NTmax = max(int(groups["nt_b"].max()), 1)
    Wmax = max(int(groups["Wmax"]), SUB)
    off = np.zeros(NB + 1, dtype=np.int64)
    off[1:] = np.cumsum(C_b)
    total_tail_alloc = max(total_tail, 1)

    nc = bacc.Bacc("TRN2")
    bf16 = mybir.dt.bfloat16
    f16 = mybir.dt.float16
    f32 = mybir.dt.float32

    g_d = nc.declare_dram_parameter("gmat", [128, total_chunks * D], bf16, isOutput=False)
    dvf_d = nc.declare_dram_parameter("dvf", [128, total_full], bf16, isOutput=False)
    dvt_d = nc.declare_dram_parameter("dvt", [128, total_tail_alloc], f16, isOutput=False)
    # misc: [0:512) wp, [512:544) iota32, partition 0 [544:800) = ones|bias
    misc_d = nc.declare_dram_parameter("misc", [128, 800], bf16, isOutput=False)
    iota32_d = nc.declare_dram_parameter("iota32", [128, SUB], bf16, isOutput=False)
    iota512_d = nc.declare_dram_parameter("iota512", [128, 512], f16, isOutput=False)
    out_d = nc.declare_dram_parameter("out", [NB * BLOCK, D], bf16, isOutput=True)

    with contextlib.ExitStack() as ctx:
        block = ctx.enter_context(nc.Block())
        g_bufs = [
            ctx.enter_context(nc.sbuf_tensor(f"g{i}", [128, Cmax * D], bf16))
            for i in range(NG_BUF)
        ]
        af_bufs = [
            ctx.enter_context(nc.sbuf_tensor(f"af{i}", [128, NFmax, SUB], bf16))
            for i in range(NA_BUF)
        ]
        at_bufs = [
            ctx.enter_context(nc.sbuf_tensor(f"at{i}", [128, NTmax, Wmax], bf16))
            for i in range(NA_BUF)
        ]
        dvf_sb = ctx.enter_context(nc.sbuf_tensor("dvf_sb", [128, total_full], bf16))
        dvt_sb = ctx.enter_context(nc.sbuf_tensor("dvt_sb", [128, total_tail_alloc], f16))
        misc_sb = ctx.enter_context(nc.sbuf_tensor("misc_sb", [128, 800], bf16))
        iota32_sb = ctx.enter_context(nc.sbuf_tensor("iota32_sb", [128, SUB], bf16))
        iota512_sb = ctx.enter_context(nc.sbuf_tensor("iota512_sb", [128, 512], f16))
        gt_sb = ctx.enter_context(nc.sbuf_tensor("gt_sb", [128, 2 * N_GRAPHS * D], bf16))
        stage = ctx.enter_context(nc.sbuf_tensor("stage", [128, 2 * D], bf16))
        gt_ps = [
            ctx.enter_context(nc.psum_tensor(f"gt{i}", [128, N_GRAPHS * D], f32))
            for i in range(4)
        ]
        # full-bank allocation (512 f32) so the two o3 buffers never share a
        # PSUM bank: PE writing o3[pb%2] while ACT reads o3[(pb-1)%2] in the
        # same bank is a hardware hazard
        o3_ps = [
            ctx.enter_context(nc.psum_tensor(f"o3{i}", [128, 512], f32))
            for i in range(2)
        ]
        io = ctx.enter_context(nc.semaphore("io"))
        g_sems = [ctx.enter_context(nc.semaphore(f"g_sem{i}")) for i in range(NG_BUF)]
        st_sems = [ctx.enter_context(nc.semaphore(f"st_sem{i}")) for i in range(2)]
        abuild_sem = ctx.enter_context(nc.semaphore("abuild_sem"))  # +1 per block
        pe_blk = ctx.enter_context(nc.semaphore("pe_blk"))  # +1 per block agg
        pe_proj = ctx.enter_context(nc.semaphore("pe_proj"))  # +1 per block proj
        act_sem = ctx.enter_context(nc.semaphore("act_sem"))  # +1 per gt copy
        o3c_sem = ctx.enter_context(nc.semaphore("o3c_sem"))  # +1 per o3 copy

        wp_ap = misc_sb[:, 0:512]
        iota32_ap = misc_sb[:, 512:544]
        ones_ap = misc_sb[0:1, 544 : 544 + D]
        bias_ap = misc_sb[0:1, 544 + D : 544 + 2 * D]

        @block.sync
        def _(sync):
            # let the small init loads (scalar ring) drain before the fat G
            # slabs monopolize the SDMA engines (packet-level round-robin
            # starves them for ~45us otherwise)
            sync.wait_ge(io, 80)
            # sync's HWDGE ring carries ONLY the big G slabs, back to back
            for b in range(NB):
                cb = int(C_b[b])
                if b >= NG_BUF:
                    sync.wait_ge(pe_blk, b - NG_BUF + 1)
                sync.dma_start(
                    g_bufs[b % NG_BUF][:, : cb * D],
                    g_d[:, int(off[b]) * D : int(off[b] + cb) * D],
                ).then_inc(g_sems[b % NG_BUF], 16)

        def _issue_store(eng, sb):
            eng.wait_ge(o3c_sem, sb + 1)
            eng.dma_start(
                out_d[BLOCK * sb : BLOCK * (sb + 1), :],
                stage[:, (sb % 2) * D : (sb % 2) * D + D],
            ).then_inc(st_sems[sb % 2], 16)

        def _proj(tensor, pb):
            # projection of block pb, deferred one block so the ACT copy
            # round-trip hides under agg(pb+1)
            for l in range(N_GRAPHS):
                if l == 0:
                    tensor.wait_ge(act_sem, pb + 1)
                if l == 0 and pb >= 2:
                    tensor.wait_ge(o3c_sem, pb - 1)  # o3 psum reuse
                tensor.matmul(
                    o3_ps[pb % 2][:, 0:D],
                    gt_sb[:, ((pb % 2) * N_GRAPHS + l) * D : ((pb % 2) * N_GRAPHS + l + 1) * D],
                    wp_ap[:, l * D : (l + 1) * D],
                    start=(l == 0),
                    stop=False,
                )
            # bias via K=1 matmul: o3 += ones^T @ bias_row
            tensor.matmul(
                o3_ps[pb % 2][:, 0:D],
                ones_ap,
                bias_ap,
                start=False,
                stop=True,
            ).then_inc(pe_proj, 1)

        @block.tensor
        def _(tensor):
            tensor.wait_ge(io, 64)
            for b in range(NB):
                tensor.wait_ge(g_sems[b % NG_BUF], 16 * (b // NG_BUF + 1))
                tensor.wait_ge(abuild_sem, b + 1)
                gbuf = g_bufs[b % NG_BUF]
                afb = af_bufs[b % NA_BUF]
                atb = at_bufs[b % NA_BUF]
                nt = int(groups["nt_b"][b])
                gt_flat = gt_ps[b % 4]
                # ONE start per bank per block: start=True marks the whole
                # 2KB zero region pending-zero, so later MMs overwrite on
                # first element touch and accumulate after — any additional
                # start would wipe earlier groups' partials for the tails.
                first = True
                last = None
                for l in range(N_GRAPHS):
                    for s in range(NSUB):
                        gidx = l * NSUB + s
                        out_ap = gt_flat[:, l * D + s * SUB : l * D + (s + 1) * SUB]
                        nf = int(groups["nfull"][b, gidx])
                        fb = int(groups["fullbase"][b, gidx]) - int(off[b])
                        for i in range(nf):
                            ci = fb + i
                            last = tensor.matmul(
                                out_ap,
                                gbuf[:, ci * D : (ci + 1) * D],
                                afb[:, ci, :],
                                start=first,
                                stop=False,
                                skip_group_check=True,
                            )
                            first = False
                for t in range(nt):
                    ci = int(groups["tailstart"][b]) - int(off[b]) + t
                    lo = int(groups["tail_lo"][b, t])
                    w = int(groups["tail_w"][b, t])
                    last = tensor.matmul(
                        gt_flat[:, lo : lo + w],
                        gbuf[:, ci * D : (ci + 1) * D],
                        atb[:, t, :w],
                        start=first,
                        stop=(t == nt - 1),
                        skip_group_check=True,
                    )
                    first = False
                last.then_inc(pe_blk, 1)
                if b >= 1:
                    _proj(tensor, b - 1)
            _proj(tensor, NB - 1)

        def _o3_copy(scalar, pb):
            # stage <- o3 psum (f32 -> bf16), after proj+bias of block pb
            scalar.wait_ge(pe_proj, pb + 1)
            if pb >= 2:
                scalar.wait_ge(st_sems[pb % 2], 16 * ((pb - 2) // 2 + 1))
            scalar.copy(
                stage[:, (pb % 2) * D : (pb % 2) * D + D],
                o3_ps[pb % 2][:, 0:D],
            ).then_inc(o3c_sem, 1)

        @block.scalar
        def _(scalar):
            # ACT owns the second HWDGE ring: init loads + stores
            scalar.dma_start(misc_sb[:, :], misc_d[:, :]).then_inc(io, 16)
            scalar.dma_start(iota512_sb[:, :], iota512_d[:, :]).then_inc(io, 16)
            scalar.dma_start(dvf_sb[:, :], dvf_d[:, :]).then_inc(io, 16)
            scalar.dma_start(dvt_sb[:, :], dvt_d[:, :]).then_inc(io, 16)
            scalar.dma_start(iota32_sb[:, :], iota32_d[:, :]).then_inc(io, 16)
            for b in range(NB):
                if b >= 2:
                    _issue_store(scalar, b - 2)
                if b >= 1:
                    _o3_copy(scalar, b - 1)
                scalar.wait_ge(pe_blk, b + 1)
                if b >= 2:
                    scalar.wait_ge(pe_proj, b - 1)  # gt_sb bank reuse
                scalar.copy(
                    gt_sb[:, (b % 2) * N_GRAPHS * D : ((b % 2) + 1) * N_GRAPHS * D],
                    gt_ps[b % 4][:, :],
                ).then_inc(act_sem, 1)
            _o3_copy(scalar, NB - 1)
            for sb in (NB - 2, NB - 1):
                _issue_store(scalar, sb)

        @block.vector
        def _(vector):
            # DVE: one-hot A build only
            vector.wait_ge(io, 80)
            fc = groups["fullcum"]
            tc = groups["tailcum"]
            for b in range(NB):
                nf = int(groups["nf_b"][b])
                nt = int(groups["nt_b"][b])
                if b >= NA_BUF:
                    vector.wait_ge(pe_blk, b - NA_BUF + 1)
                afb = af_bufs[b % NA_BUF]
                atb = at_bufs[b % NA_BUF]
                dvf_b = dvf_sb[:, int(fc[b]) : int(fc[b]) + nf]
                a3 = afb[:, :nf, :]
                ieq = vector.tensor_tensor(
                    a3,
                    dvf_b.unsqueeze(2).broadcast_to([128, nf, SUB]),
                    iota32_sb[:, :].unsqueeze(1).broadcast_to([128, nf, SUB]),
                    mybir.AluOpType.is_equal,
                )
                if nt > 0:
                    dvt_b = dvt_sb[:, int(tc[b]) : int(tc[b]) + nt]
                    t3 = atb[:, :nt, :]
                    ieq = vector.tensor_tensor(
                        t3,
                        dvt_b.unsqueeze(2).broadcast_to([128, nt, Wmax]),
                        iota512_sb[:, :Wmax].unsqueeze(1).broadcast_to([128, nt, Wmax]),
                        mybir.AluOpType.is_equal,
                    )
                ieq.then_inc(abuild_sem, 1)

    nc.compile()
    return nc


_TRACE = {"on": False, "last": None}


def kernel(x, edge_rows, edge_cols, edge_vals, W, mixing_weight, bias):
    from concourse.bass_utils import run_bass_kernel_spmd

    sched = _host_schedule(x, edge_rows, edge_cols, edge_vals)
    nc = _build_nc(
        sched["groups"],
        sched["C_b"],
        sched["total_chunks"],
        sched["total_full"],
        sched["total_tail"],
    )

    Wp = (np.asarray(mixing_weight, dtype=np.float32)[:, 0, None, None]
          * np.asarray(W, dtype=np.float32))  # [4,128,128]
    misc = np.zeros((128, 800), dtype=ml_dtypes.bfloat16)
    misc[:, 0:512] = np.ascontiguousarray(
        np.transpose(Wp, (1, 0, 2)).reshape(D, N_GRAPHS * D)
    ).astype(ml_dtypes.bfloat16)
    misc[:, 512:544] = np.broadcast_to(
        np.arange(SUB, dtype=np.float32), (128, SUB)
    ).astype(ml_dtypes.bfloat16)
    misc[0, 544 : 544 + D] = np.ones(D, dtype=np.float32).astype(ml_dtypes.bfloat16)
    misc[0, 544 + D : 544 + 2 * D] = np.asarray(bias, dtype=np.float32).astype(
        ml_dtypes.bfloat16
    )
    iota512 = np.ascontiguousarray(
        np.broadcast_to(np.arange(512, dtype=np.float32), (128, 512))
    ).astype(np.float16)

    in_maps = [
        {
            "gmat": sched["g_arrs"][s],
            "dvf": sched["dvf_arrs"][s],
            "dvt": sched["dvt_arrs"][s],
            "misc": misc,
            "iota512": iota512,
            "iota32": np.ascontiguousarray(
                np.broadcast_to(np.arange(SUB, dtype=np.float32), (128, SUB))
            ).astype(ml_dtypes.bfloat16),
        }
        for s in range(N_CORES)
    ]

    res = run_bass_kernel_spmd(
        nc, in_maps, core_ids=list(range(N_CORES)), trace=_TRACE["on"]
    )
    _TRACE["last"] = res
    out = np.concatenate(
        [
            np.asarray(res.results[s]["out"][:ROWS_PER_CORE], dtype=np.float32)
            for s in range(N_CORES)
        ],
        axis=0,
    )
    return out
